# revision 1
# baseline (speedup 1.0000x reference)
"""Self-attention (1x1-conv QKV projections + NxN softmax attention + residual)
for x:(4,256,64,64) on 8 TRN2 NeuronCores.

Sharding: core = 2*b + h  ->  batch b in 0..3, query-half h in 0..1.
Each core computes out[b][:, h*2048:(h+1)*2048] (softmax is row-wise over
keys, so splitting query rows is embarrassingly parallel).

Per-core kernel (all matmuls float32r = PE fast-fp32 mode, 1 cycle/row):
  k_sb (32,4096) = Wk@x + bk, q_sb (32,2048) = Wq@x[:,msl] + bq
  v_sb (128,32,256): v^T tiles, v_T[n,c] = sum_c' x[c',n] WvT[c',c]
  energy (transposed, per key-tile pair): e[n,m] = sum_d k[d,n] q[d,m]
    -> (128,1024) PSUM pairs, double-buffered
  p = exp(e / sqrt(32))    (no max subtraction: |e*s| <~ 6, exp is <=2ulp)
  out[c,m] = sum_n v_T[n,c] p[n,m]   (K=128 full-array matmuls, PSUM-accum)
  rowsum[m] = sum_n p[n,m] via ones-lhsT matmuls accumulating on one bank
  final: out = out_raw / rowsum + bv + x_residual

k/q/v projections are interleaved per 512-column x-chunk so the PE starts
as soon as the first input DMA lands instead of waiting for all of x.
"""

import numpy as np

B, C, N = 4, 256, 4096
DK = 32
MH = N // 2          # 2048 query rows per core
NT = N // 128        # 32 key tiles
SBW = 512            # query superblock width
SCALE = 1.0 / float(np.sqrt(DK))

_cache = {}


def _build_nc():
    from contextlib import ExitStack

    import concourse.bacc as bacc
    import concourse.bass as bass
    import concourse.mybir as mybir
    import concourse.tile as tile

    f32 = mybir.dt.float32
    f32r = mybir.dt.float32r
    Exp = mybir.ActivationFunctionType.Exp
    add = mybir.AluOpType.add
    mult = mybir.AluOpType.mult

    nc = bacc.Bacc("TRN2", target_bir_lowering=False, debug=False)

    xf_d = nc.dram_tensor("xf", [C, N], f32r, kind="ExternalInput").ap()
    xq_d = nc.dram_tensor("xq", [C, MH], f32r, kind="ExternalInput").ap()
    wqt_d = nc.dram_tensor("wqt", [C, DK], f32r, kind="ExternalInput").ap()
    wkt_d = nc.dram_tensor("wkt", [C, DK], f32r, kind="ExternalInput").ap()
    wvt_d = nc.dram_tensor("wvt", [C, C], f32r, kind="ExternalInput").ap()
    bq_d = nc.dram_tensor("bq", [DK, 1], f32, kind="ExternalInput").ap()
    bk_d = nc.dram_tensor("bk", [DK, 1], f32, kind="ExternalInput").ap()
    bv_d = nc.dram_tensor("bv", [C, 1], f32, kind="ExternalInput").ap()
    ones_d = nc.dram_tensor("ones32", [128, DK], f32r, kind="ExternalInput").ap()
    out_d = nc.dram_tensor("out", [C, MH], f32, kind="ExternalOutput").ap()

    with tile.TileContext(nc) as tc, ExitStack() as ctx:
        const = ctx.enter_context(tc.tile_pool(name="const", bufs=1))

        # ---- weights / biases to SBUF ----
        wqt_sb = const.tile([128, 2, DK], f32r)
        wkt_sb = const.tile([128, 2, DK], f32r)
        wvt_sb = const.tile([128, 2, C], f32r)

        def split_c(dram_ap, width):
            # (256, width) -> stream (p, a, m) matching a [128, 2, width] tile
            return bass.AP(tensor=dram_ap.tensor, offset=dram_ap.offset,
                           ap=[[width, 128], [128 * width, 2], [1, width]])

        nc.sync.dma_start(out=wqt_sb, in_=split_c(wqt_d, DK))
        nc.sync.dma_start(out=wkt_sb, in_=split_c(wkt_d, DK))

        # ---- inputs: xq first (q unblocks the main loop), then x chunks ----
        x_sb = const.tile([128, 2, N], f32r)
        xq_sb = const.tile([128, 2, MH], f32r)

        def chunk_c(dram_ap, width, j, cw):
            return bass.AP(tensor=dram_ap.tensor, offset=dram_ap.offset + j * cw,
                           ap=[[width, 128], [128 * width, 2], [1, cw]])

        for j in range(4):
            nc.sync.dma_start(out=xq_sb[:, :, j * 512:(j + 1) * 512],
                              in_=chunk_c(xq_d, MH, j, 512))
        bq_sb = const.tile([DK, 1], f32)
        bk_sb = const.tile([DK, 1], f32)
        bv_sb = const.tile([128, 2], f32)
        nc.sync.dma_start(out=bq_sb, in_=bq_d)
        nc.sync.dma_start(out=bk_sb, in_=bk_d)
        for a in range(2):
            nc.sync.dma_start(out=bv_sb[:, a:a + 1], in_=bv_d[a * 128:(a + 1) * 128, :])

        ones = const.tile([128, DK], f32r)
        nc.sync.dma_start(out=ones, in_=ones_d)

        nc.sync.dma_start(out=wvt_sb, in_=split_c(wvt_d, C))
        for j in range(8):
            nc.sync.dma_start(out=x_sb[:, :, j * 512:(j + 1) * 512],
                              in_=chunk_c(xf_d, N, j, 512))

        k_sb = const.tile([DK, N], f32r)
        q_sb = const.tile([DK, MH], f32r)
        v_sb = const.tile([128, NT, C], f32r)

        # ---- projections, interleaved per x-chunk ----
        with tc.tile_pool(name="proj_ps", bufs=2, space="PSUM") as pp:
            for j in range(MH // SBW):
                qp = pp.tile([DK, SBW], f32, name="qp", tag="qp")
                for a in range(2):
                    nc.tensor.matmul(
                        qp, wqt_sb[:, a, :],
                        xq_sb[:, a, j * SBW:(j + 1) * SBW],
                        start=(a == 0), stop=(a == 1))
                nc.vector.tensor_scalar_add(
                    q_sb[:, j * SBW:(j + 1) * SBW], qp, bq_sb)
            for j in range(N // SBW):
                kp = pp.tile([DK, SBW], f32, name="kp", tag="kp")
                for a in range(2):
                    nc.tensor.matmul(
                        kp, wkt_sb[:, a, :],
                        x_sb[:, a, j * SBW:(j + 1) * SBW],
                        start=(a == 0), stop=(a == 1))
                nc.vector.tensor_scalar_add(
                    k_sb[:, j * SBW:(j + 1) * SBW], kp, bk_sb)
                for t in range(4 * j, 4 * j + 4):
                    vp = pp.tile([128, C], f32, name="vp", tag="vp")
                    for a in range(2):
                        nc.tensor.matmul(
                            vp,
                            x_sb[:, a, t * 128:(t + 1) * 128],
                            wvt_sb[:, a, :],
                            start=(a == 0), stop=(a == 1))
                    nc.vector.tensor_copy(out=v_sb[:, t, :], in_=vp)

        # ---- main attention loop: 16 key-tile pairs per query superblock ----
        ep = ctx.enter_context(tc.tile_pool(name="e_ps", bufs=2, space="PSUM"))
        op = ctx.enter_context(tc.tile_pool(name="o_ps", bufs=1, space="PSUM"))
        rp = ctx.enter_context(tc.tile_pool(name="rs_ps", bufs=2, space="PSUM"))
        ppool = ctx.enter_context(tc.tile_pool(name="p_sb", bufs=3))
        misc = ctx.enter_context(tc.tile_pool(name="misc", bufs=2))
        outp = ctx.enter_context(tc.tile_pool(name="outp", bufs=2))

        for sbk in range(MH // SBW):
            msl = slice(sbk * SBW, (sbk + 1) * SBW)
            o_ps = [op.tile([128, SBW], f32, name=f"o_ps{c}", tag=f"o_ps{c}")
                    for c in range(2)]
            rs_ps = rp.tile([DK, SBW], f32)
            # 1-stage software pipeline: emit energy(pr) ahead of PV(pr-1)
            # so the PE FIFO never head-of-line blocks on exp(pr-1).
            pend = None
            for pr in range(NT // 2 + 1):
                if pr < NT // 2:
                    e_pair = ep.tile([128, 2 * SBW], f32,
                                     name="e_pair", tag="e_pair")
                    for i in range(2):
                        t = 2 * pr + i
                        nc.tensor.matmul(
                            e_pair[:, i * SBW:(i + 1) * SBW],
                            k_sb[:, t * 128:(t + 1) * 128],
                            q_sb[:, msl],
                            start=True, stop=True)
                if pend is not None:
                    p_prev, pr_prev = pend
                    for i in range(2):
                        t = 2 * pr_prev + i
                        prhs = p_prev[:, i * SBW:(i + 1) * SBW]
                        for c in range(2):
                            nc.tensor.matmul(
                                o_ps[c],
                                v_sb[:, t, c * 128:(c + 1) * 128],
                                prhs,
                                start=(t == 0), stop=(t == NT - 1))
                        nc.tensor.matmul(
                            rs_ps, ones, prhs,
                            start=(t == 0), stop=(t == NT - 1))
                if pr < NT // 2:
                    p_pair = ppool.tile([128, 2 * SBW], f32r,
                                        name="p_pair", tag="p_pair")
                    nc.scalar.activation(p_pair, e_pair, Exp, scale=SCALE)
                    pend = (p_pair, pr)

            # softmax denominator: partitions 0-31 of rs_ps all hold rowsum
            rec = misc.tile([1, SBW], f32)
            nc.vector.reciprocal(out=rec, in_=rs_ps[0:1, :])
            rec_rep = misc.tile([128, SBW], f32)
            nc.gpsimd.partition_broadcast(rec_rep, rec)

            for c in range(2):
                osb = outp.tile([128, SBW], f32, name=f"osb{c}", tag=f"osb{c}")
                nc.vector.scalar_tensor_tensor(
                    out=osb, in0=o_ps[c], scalar=0.0, in1=rec_rep,
                    op0=add, op1=mult)
                ofin = outp.tile([128, SBW], f32, name=f"ofin{c}", tag=f"ofin{c}")
                nc.vector.scalar_tensor_tensor(
                    out=ofin, in0=osb, scalar=bv_sb[:, c:c + 1],
                    in1=xq_sb[:, c, msl].bitcast(f32), op0=add, op1=add)
                nc.sync.dma_start(out=out_d[c * 128:(c + 1) * 128, msl], in_=ofin)

    nc.compile()
    return nc


def kernel(x, Wq, bq, Wk, bk, Wv, bv):
    from concourse import bass_utils

    x = np.asarray(x, np.float32)
    xf = np.ascontiguousarray(x.reshape(B, C, N))
    wqt = np.ascontiguousarray(np.asarray(Wq, np.float32).T)
    wkt = np.ascontiguousarray(np.asarray(Wk, np.float32).T)
    wvt = np.ascontiguousarray(np.asarray(Wv, np.float32).T)
    bq2 = np.ascontiguousarray(np.asarray(bq, np.float32).reshape(DK, 1))
    bk2 = np.ascontiguousarray(np.asarray(bk, np.float32).reshape(DK, 1))
    bv2 = np.ascontiguousarray(np.asarray(bv, np.float32).reshape(C, 1))
    ones32 = np.ones((128, DK), np.float32)

    if "nc" not in _cache:
        _cache["nc"] = _build_nc()
    nc = _cache["nc"]

    in_maps = []
    for core in range(8):
        b, h = core // 2, core % 2
        in_maps.append({
            "xf": xf[b],
            "xq": np.ascontiguousarray(xf[b][:, h * MH:(h + 1) * MH]),
            "wqt": wqt, "wkt": wkt, "wvt": wvt,
            "bq": bq2, "bk": bk2, "bv": bv2,
            "ones32": ones32,
        })

    res = bass_utils.run_bass_kernel_spmd(nc, in_maps, core_ids=list(range(8)))
    out = np.empty((B, C, N), np.float32)
    for core in range(8):
        b, h = core // 2, core % 2
        out[b][:, h * MH:(h + 1) * MH] = res.results[core]["out"]
    return out.reshape(B, C, 64, 64)



# revision 20
# speedup vs baseline: 1.5141x; 1.5141x over previous
"""Self-attention (1x1-conv QKV projections + NxN softmax attention + residual)
for x:(4,256,64,64) on 8 TRN2 NeuronCores — fp8 DoubleRow edition.

Sharding: core = 2*b + h -> batch b in 0..3, query-half h in 0..1.
Each core computes out[b][:, h*2048:(h+1)*2048].

All matmuls run in fp8(e4m3) DoubleRow perf mode (0.5 cycles/row, 2x the
fp32r rate, contracting 256 deep per instruction):
  proj:  q' = (16Wq)@x8 (DR over the two 128-channel halves), same k', v'
         (the x16 keeps the N(0,1/256) weights out of fp8 subnormals;
          all later stages are scale-consistent so the 16 cancels)
  q'/k' are re-packed to [16,2,*] via SBUF->SBUF DMA so the d=32
         contraction also runs DoubleRow.
  energy e' = k'^T q' = 256*q.k  -> z = e' / (256*sqrt(32))
  p8 = exp(z - ln32)  (downscale keeps exp < fp8-e4m3 max 240; the
       softmax division cancels it).  exp is split across engines:
       Act-engine native Exp for most key-tile pairs, DVE Schraudolph
       (int32 bit-trick exp, 2 ops, one at 2x_2p rate) for the rest —
       the Act engine alone (1 elem/cycle/partition @1.2GHz) would be
       the bottleneck at ~55us.
  PV:   out' += v'_tiles^T p8 (DR), rowsum += ones^T p8 (DR)
  tail: out = out'/(16*rowsum) + bv + x_residual
"""

import numpy as np

B, C, N = 4, 256, 4096
DK = 32
MH = N // 2          # 2048 query rows per core
NT = N // 128        # 32 key tiles -> 16 DoubleRow pairs
SBW = 512            # query superblock width
NPR = NT // 2        # 16 key-tile pairs per superblock

LN32 = float(np.log(32.0))
S_ACT = 1.0 / (256.0 * float(np.sqrt(DK)))       # z = e' * S_ACT
A_SCH = 2.0 ** 23 / float(np.log(2.0))           # Schraudolph slope
B_SCH = 1064990000.0                             # tuned for int32 trunc
SC1 = A_SCH * S_ACT                              # int = e'*SC1 + SC2
SC2 = B_SCH - A_SCH * LN32

# which key-tile pairs (by index within a superblock) take the DVE
# Schraudolph path instead of the Act-engine Exp.  NOTE the e-buffer ring
# (bufs=2) makes pairs at distance 2 a serial chain, so DVE pairs are
# spread across both parities.  GPSIMD cannot read PSUM on real HW, so
# only Act/DVE can touch e_pair; the Pool engine gets the SBUF->SBUF
# fp8-quantize (ts2) of the Schraudolph pairs instead.
DVE_PRS = (2, 5, 8, 11, 14)

_cache = {}


def _build_nc():
    from contextlib import ExitStack

    import concourse.bacc as bacc
    import concourse.bass as bass
    import concourse.mybir as mybir
    import concourse.tile as tile

    f32 = mybir.dt.float32
    i32 = mybir.dt.int32
    u8 = mybir.dt.uint8
    fp8 = mybir.dt.float8e4
    Exp = mybir.ActivationFunctionType.Exp
    DR = mybir.MatmulPerfMode.DoubleRow
    add = mybir.AluOpType.add
    mult = mybir.AluOpType.mult

    nc = bacc.Bacc("TRN2", target_bir_lowering=False, debug=False)

    # fp8 payloads travel as uint8 and are bitcast on-chip (avoids any
    # PJRT f8e4m3 dtype plumbing issues)
    x8_d = nc.dram_tensor("x8", [128, 2, N], u8, kind="ExternalInput").ap()
    xq8_d = nc.dram_tensor("xq8", [128, 2, MH], u8, kind="ExternalInput").ap()
    xq_d = nc.dram_tensor("xq", [128, 2, MH], f32, kind="ExternalInput").ap()
    wqt_d = nc.dram_tensor("wqt8", [128, 2, DK], u8, kind="ExternalInput").ap()
    wkt_d = nc.dram_tensor("wkt8", [128, 2, DK], u8, kind="ExternalInput").ap()
    wvt_d = nc.dram_tensor("wvt8", [128, 2, C], u8, kind="ExternalInput").ap()
    ones_d = nc.dram_tensor("ones8", [128, 2, DK], u8, kind="ExternalInput").ap()
    bq_d = nc.dram_tensor("bq16", [DK, 1], f32, kind="ExternalInput").ap()
    bk_d = nc.dram_tensor("bk16", [DK, 1], f32, kind="ExternalInput").ap()
    bv_d = nc.dram_tensor("bv", [C, 1], f32, kind="ExternalInput").ap()
    out_d = nc.dram_tensor("out", [C, MH], f32, kind="ExternalOutput").ap()

    with tile.TileContext(nc) as tc, ExitStack() as ctx:
        const = ctx.enter_context(tc.tile_pool(name="const", bufs=1))

        wqt_sb = const.tile([128, 2, DK], u8)
        wkt_sb = const.tile([128, 2, DK], u8)
        wvt_sb = const.tile([128, 2, C], u8)
        ones_sb = const.tile([128, 2, DK], u8)
        nc.sync.dma_start(out=wqt_sb, in_=wqt_d)
        nc.sync.dma_start(out=wkt_sb, in_=wkt_d)
        nc.sync.dma_start(out=wvt_sb, in_=wvt_d)
        nc.sync.dma_start(out=ones_sb, in_=ones_d)

        ebias_sb = const.tile([128, 1], f32)
        nc.gpsimd.memset(ebias_sb, -LN32)

        bq_sb = const.tile([DK, 1], f32)
        bk_sb = const.tile([DK, 1], f32)
        bv_sb = const.tile([128, 2], f32)
        nc.sync.dma_start(out=bq_sb, in_=bq_d)
        nc.sync.dma_start(out=bk_sb, in_=bk_d)
        nc.sync.dma_start(out=bv_sb,
                          in_=bass.AP(tensor=bv_d.tensor, offset=bv_d.offset,
                                      ap=[[1, 128], [128, 2]]))

        x8_sb = const.tile([128, 2, N], u8)
        xq8_sb = const.tile([128, 2, MH], u8)
        xq_sb = const.tile([128, 2, MH], f32)

        def chunk3(dram_ap, width, j, cw):
            # [:, :, j*cw:(j+1)*cw] of a [128, 2, width] dram tensor
            return bass.AP(tensor=dram_ap.tensor, offset=dram_ap.offset + j * cw,
                           ap=[[2 * width, 128], [width, 2], [1, cw]])

        # query-half fp8 first: it unblocks the q projection & repack.
        # Batched coarsely -- every DMA pays ~0.6us on the shared HWDGE.
        for j in range(2):
            nc.sync.dma_start(out=xq8_sb[:, :, j * 1024:(j + 1) * 1024],
                              in_=chunk3(xq8_d, MH, j, 1024))
        for j in range(4):
            nc.sync.dma_start(out=x8_sb[:, :, j * 1024:(j + 1) * 1024],
                              in_=chunk3(x8_d, N, j, 1024))
        # residual (f32) rides the Act-triggered HW queue so it never blocks
        # the fp8 input stream on the SP queue
        for j in range(2):
            nc.scalar.dma_start(out=xq_sb[:, :, j * 1024:(j + 1) * 1024],
                                in_=chunk3(xq_d, MH, j, 1024))

        x8f = x8_sb.bitcast(fp8)
        xq8f = xq8_sb.bitcast(fp8)
        wq8 = wqt_sb.bitcast(fp8)
        wk8 = wkt_sb.bitcast(fp8)
        wv8 = wvt_sb.bitcast(fp8)

        q8tmp = const.tile([DK, MH], u8)
        k8tmp = const.tile([DK, N], u8)
        q8_sb = const.tile([16, 2, MH], u8)
        k8_sb = const.tile([16, 2, N], u8)
        v8_sb = const.tile([128, NT, C], u8)

        # ---- projections (all DoubleRow) ----
        with tc.tile_pool(name="proj_ps", bufs=2, space="PSUM") as pp:
            for j in range(MH // 512):
                qp = pp.tile([DK, 512], f32, name="qp", tag="qp")
                nc.tensor.matmul(qp, wq8,
                                 xq8f[:, :, j * 512:(j + 1) * 512],
                                 start=True, stop=True, perf_mode=DR)
                nc.vector.tensor_scalar_add(
                    q8tmp.bitcast(fp8)[:, j * 512:(j + 1) * 512], qp, bq_sb)
                for i in range(2):
                    nc.sync.dma_start(
                        out=q8_sb[:, i, j * 512:(j + 1) * 512],
                        in_=q8tmp[16 * i:16 * (i + 1), j * 512:(j + 1) * 512])
            for j in range(N // 512):
                kp = pp.tile([DK, 512], f32, name="kp", tag="kp")
                nc.tensor.matmul(kp, wk8,
                                 x8f[:, :, j * 512:(j + 1) * 512],
                                 start=True, stop=True, perf_mode=DR)
                nc.vector.tensor_scalar_add(
                    k8tmp.bitcast(fp8)[:, j * 512:(j + 1) * 512], kp, bk_sb)
                for i in range(2):
                    nc.sync.dma_start(
                        out=k8_sb[:, i, j * 512:(j + 1) * 512],
                        in_=k8tmp[16 * i:16 * (i + 1), j * 512:(j + 1) * 512])
                # v tiles for this chunk: 4 tiles, 2 per PSUM buffer
                for half in range(2):
                    vp = pp.tile([128, 2, C], f32, name="vp", tag="vp")
                    for tt in range(2):
                        t = 4 * j + 2 * half + tt
                        nc.tensor.matmul(
                            vp[:, tt, :],
                            x8f[:, :, t * 128:(t + 1) * 128],
                            wv8, start=True, stop=True, perf_mode=DR)
                    # PSUM->SBUF quantizing copy; alternate Act/Pool engines
                    dst = v8_sb.bitcast(fp8)[:, 4 * j + 2 * half:
                                             4 * j + 2 * half + 2, :]
                    if (2 * j + half) % 4 < 3:
                        nc.scalar.activation(dst, vp,
                                             mybir.ActivationFunctionType.Copy)
                    else:
                        nc.vector.tensor_scalar_add(dst, vp, 0.0)

        q8f = q8_sb.bitcast(fp8)
        k8f = k8_sb.bitcast(fp8)
        v8f = v8_sb.bitcast(fp8)
        o8f = ones_sb.bitcast(fp8)

        # ---- main attention loop ----
        ep = ctx.enter_context(tc.tile_pool(name="e_ps", bufs=2, space="PSUM"))
        op = ctx.enter_context(tc.tile_pool(name="o_ps", bufs=1, space="PSUM"))
        rp = ctx.enter_context(tc.tile_pool(name="rs_ps", bufs=2, space="PSUM"))
        ppool = ctx.enter_context(tc.tile_pool(name="p_sb", bufs=3))
        ipool = ctx.enter_context(tc.tile_pool(name="i32_sb", bufs=2))
        misc = ctx.enter_context(tc.tile_pool(name="misc", bufs=2))
        outp = ctx.enter_context(tc.tile_pool(name="outp", bufs=2))

        def do_exp(e_pair, p_pair, pr):
            if pr in DVE_PRS:
                # Schraudolph exp: int32 bit-trick on DVE (reads PSUM),
                # then SBUF->SBUF fp8 quantize on the idle Pool engine
                tI = ipool.tile([128, 2, SBW], i32, name="tI", tag="tI")
                nc.vector.tensor_scalar(tI, e_pair, SC1, SC2,
                                        op0=mult, op1=add)
                nc.vector.tensor_scalar_add(p_pair, tI.bitcast(f32), 0.0)
            else:
                nc.scalar.activation(p_pair, e_pair, Exp,
                                     bias=ebias_sb, scale=S_ACT)

        for sbk in range(MH // SBW):
            msl = slice(sbk * SBW, (sbk + 1) * SBW)
            o_ps = [op.tile([128, SBW], f32, name=f"o_ps{c}", tag=f"o_ps{c}")
                    for c in range(2)]
            rs_ps = rp.tile([DK, SBW], f32)
            # 2-stage software pipeline: energy runs 2 pairs ahead of PV so
            # the Act/DVE/Pool exps of consecutive pairs overlap (PE executes
            # in order; PV(pr) stalling on exp(pr) must not block energy).
            pend = []
            for pr in range(NPR + 2):
                if pr < NPR:
                    e_pair = ep.tile([128, 2, SBW], f32,
                                     name="e_pair", tag="e_pair")
                    for i in range(2):
                        t = 2 * pr + i
                        nc.tensor.matmul(
                            e_pair[:, i, :],
                            k8f[:, :, t * 128:(t + 1) * 128],
                            q8f[:, :, msl],
                            start=True, stop=True, perf_mode=DR)
                    p_pair = ppool.tile([128, 2, SBW], fp8,
                                        name="p_pair", tag="p_pair")
                    do_exp(e_pair, p_pair, pr)
                    pend.append((p_pair, pr))
                if len(pend) > 2 or (pr >= NPR and pend):
                    p_prev, pr_prev = pend.pop(0)
                    st = pr_prev == 0
                    sp = pr_prev == NPR - 1
                    for c in range(2):
                        nc.tensor.matmul(
                            o_ps[c],
                            v8f[:, 2 * pr_prev:2 * pr_prev + 2,
                                c * 128:(c + 1) * 128],
                            p_prev, start=st, stop=sp, perf_mode=DR)
                    nc.tensor.matmul(rs_ps, o8f, p_prev,
                                     start=st, stop=sp, perf_mode=DR)

            # ---- epilogue: drain o_ps fast (frees the PSUM banks for the
            # next superblock), then normalize off the critical path ----
            osb = [outp.tile([128, SBW], f32, name=f"osb{c}", tag=f"osb{c}")
                   for c in range(2)]
            for c in range(2):
                nc.scalar.activation(osb[c], o_ps[c],
                                     mybir.ActivationFunctionType.Copy)
            rec = misc.tile([1, SBW], f32)
            nc.vector.reciprocal(out=rec, in_=rs_ps[0:1, :])
            rec_rep = misc.tile([128, SBW], f32)
            nc.gpsimd.partition_broadcast(rec_rep, rec)

            for c in range(2):
                on = outp.tile([128, SBW], f32, name=f"on{c}", tag=f"on{c}")
                nc.vector.scalar_tensor_tensor(
                    out=on, in0=osb[c], scalar=1.0 / 16.0, in1=rec_rep,
                    op0=mult, op1=mult)
                ofin = outp.tile([128, SBW], f32, name=f"ofin{c}", tag=f"ofin{c}")
                nc.vector.scalar_tensor_tensor(
                    out=ofin, in0=on, scalar=bv_sb[:, c:c + 1],
                    in1=xq_sb[:, c, msl], op0=add, op1=add)
                nc.scalar.dma_start(out=out_d[c * 128:(c + 1) * 128, msl],
                                    in_=ofin)

    nc.compile()
    return nc


def kernel(x, Wq, bq, Wk, bk, Wv, bv):
    import ml_dtypes
    from concourse import bass_utils

    FP8 = ml_dtypes.float8_e4m3

    x = np.asarray(x, np.float32)
    xf = x.reshape(B, C, N)

    def to8(a):
        return np.ascontiguousarray(a.astype(FP8).view(np.uint8))

    def as3d(a2d, width):
        # (C, width) -> (128, 2, width) with c = a*128 + c_lo
        return np.ascontiguousarray(a2d.reshape(2, 128, width).transpose(1, 0, 2))

    wqt8 = to8(as3d(16.0 * np.asarray(Wq, np.float32).T.reshape(C, DK), DK))
    wkt8 = to8(as3d(16.0 * np.asarray(Wk, np.float32).T.reshape(C, DK), DK))
    wvt8 = to8(as3d(16.0 * np.asarray(Wv, np.float32).T.reshape(C, C), C))
    ones8 = to8(np.ones((128, 2, DK), np.float32))
    bq16 = np.ascontiguousarray(16.0 * np.asarray(bq, np.float32).reshape(DK, 1))
    bk16 = np.ascontiguousarray(16.0 * np.asarray(bk, np.float32).reshape(DK, 1))
    bv2 = np.ascontiguousarray(np.asarray(bv, np.float32).reshape(C, 1))

    in_maps = []
    for core in range(8):
        b, h = core // 2, core % 2
        x3 = as3d(xf[b], N)
        x8 = to8(x3)
        in_maps.append({
            "x8": x8,
            "xq8": np.ascontiguousarray(x8[:, :, h * MH:(h + 1) * MH]),
            "xq": np.ascontiguousarray(x3[:, :, h * MH:(h + 1) * MH]),
            "wqt8": wqt8, "wkt8": wkt8, "wvt8": wvt8, "ones8": ones8,
            "bq16": bq16, "bk16": bk16, "bv": bv2,
        })

    if "nc" not in _cache:
        _cache["nc"] = _build_nc()
    nc = _cache["nc"]

    res = bass_utils.run_bass_kernel_spmd(nc, in_maps, core_ids=list(range(8)))
    out = np.empty((B, C, N), np.float32)
    for core in range(8):
        b, h = core // 2, core % 2
        out[b][:, h * MH:(h + 1) * MH] = res.results[core]["out"]
    return out.reshape(B, C, 64, 64)
